# revision 9
# baseline (speedup 1.0000x reference)
"""Trainium2 Bass kernel for nn_AttentionHead_Hybrid1 (quantum-inspired attention head).

Computation (per batch b of a [B=64, S=1024, D=64] input):
    n_i   = ||x_i||;  u_i = x_i / n_i
    W     = givens_orthogonal(phi)                (tiny, sequential -> host)
    A     = (n_i n_j) (u_i^T W^T u_j)^2           (S x S scores)
    V     = x Wv^T + bv
    out   = LayerNorm(softmax(A/sqrt(D)) V + x)

Kernel strategy (data-parallel over batch, 8 batches per NeuronCore):
  * Fold norms into the score matmul:  ut_i = x_i / sqrt(n_i),
    G^T = (ut W) ut^T, so A^T = G^T * G^T elementwise.
  * Scores computed directly in transposed [j, i] layout so the PV matmul
    needs no transpose of the softmax matrix.
  * Softmax denominator comes out of the PV matmul via an appended
    all-ones column of V (the column materializes automatically from the
    bias-fold trick below).
  * No max-subtraction needed: max exponent of A/sqrt(D) is ~4.
  * The softmax division is eliminated entirely: LayerNorm is invariant to
    per-row scaling, so we feed it h' = (P V) + denom * x and scale the
    variance epsilon by denom^2.
  * 1/sqrt and x^(+-1/4) are computed as exp(c*ln(x)) so the only ACT table
    set used in the whole kernel is natural_log_exp_and_others.
  * All large matmuls run as float32r (full PE rate at N=512).
"""

import math
import os
import sys

import numpy as np

sys.path.insert(0, "/opt/trn_rl_repo")

import concourse.bass as bass
import concourse.bacc as bacc
import concourse.tile as tile
from concourse import mybir
from concourse.bass_utils import run_bass_kernel_spmd

F32 = mybir.dt.float32
F32R = mybir.dt.float32r
BF16 = mybir.dt.bfloat16
AX = mybir.AxisListType.X
OP = mybir.AluOpType
AF = mybir.ActivationFunctionType

B, S, D = 64, 1024, 64
NCORES = 8
NB = B // NCORES          # batches per core
NC = S // 128             # 128-row chunks per batch
DA = D + 1                # V augmented with ones column
INV_SQRT_D = 1.0 / math.sqrt(D)
LN_EPS = 1e-5

# score chunks whose squaring runs on ScalarE instead of VectorE (load balance)
ACT_SQUARE_CHUNKS = frozenset({5, 6, 7})


def _givens_orthogonal(phi: np.ndarray, d: int) -> np.ndarray:
    pairs = [(i, i + 1) for i in range(d - 1)] + [(i, i + 1) for i in range(d - 3, -1, -1)]
    W = np.eye(d, dtype=np.float64)
    p = phi.astype(np.float64)
    for k, (i, j) in enumerate(pairs):
        c, s = np.cos(p[k]), np.sin(p[k])
        ri, rj = W[i].copy(), W[j].copy()
        W[i] = c * ri + s * rj
        W[j] = -s * ri + c * rj
    return W.astype(np.float32)


def _build_nc() -> bass.Bass:
    nc = bacc.Bacc("TRN2", target_bir_lowering=False, debug=False, num_devices=NCORES)

    x_d = nc.dram_tensor("x", [NB, S, D], F32, kind="ExternalInput").ap()
    w_d = nc.dram_tensor("wg", [D, D], F32, kind="ExternalInput").ap()
    wv_d = nc.dram_tensor("wv_aug", [DA, DA], F32, kind="ExternalInput").ap()
    id_d = nc.dram_tensor("ident", [128, 128], F32, kind="ExternalInput").ap()
    out_d = nc.dram_tensor("out", [NB, S, D], F32, kind="ExternalOutput").ap()

    with tile.TileContext(nc) as tc:
        with (
            tc.tile_pool(name="const", bufs=1) as constp,
            tc.tile_pool(name="xin", bufs=2) as xin,
            tc.tile_pool(name="prep", bufs=2) as prep,
            tc.tile_pool(name="stats", bufs=2) as stats,
            tc.tile_pool(name="big", bufs=2) as big,
            tc.tile_pool(name="score", bufs=3) as score,
            tc.tile_pool(name="lnp", bufs=2) as lnp,
            tc.tile_pool(name="pg", bufs=2, space="PSUM") as pgp,
            tc.tile_pool(name="po", bufs=1, space="PSUM") as pop,
            tc.tile_pool(name="psm", bufs=2, space="PSUM") as psm,
        ):
            w_sb = constp.tile([D, D], F32)
            nc.sync.dma_start(w_sb, w_d)
            wv_sb = constp.tile([DA, DA], F32)
            nc.sync.dma_start(wv_sb, wv_d)
            ident = constp.tile([128, 128], F32)
            nc.sync.dma_start(ident, id_d)
            w_r = constp.tile([D, D], F32R)
            nc.vector.tensor_copy(w_r, w_sb)

            for b in range(NB):
                # ---- load x[b] as [128, NC, 64] --------------------------------
                x_sb = xin.tile([128, NC, D], F32)
                nc.sync.dma_start(x_sb, x_d[b].rearrange("(c p) d -> p c d", p=128))

                # ---- norms: nsq, s = nsq^-1/4, rtn = nsq^+1/4 ------------------
                xsq = prep.tile([128, NC, D], F32, tag="xsq")
                nc.vector.tensor_mul(xsq, x_sb, x_sb)
                nsq = stats.tile([128, NC], F32, tag="nsq")
                nc.vector.reduce_sum(nsq, xsq, axis=AX)
                lnn = stats.tile([128, NC], F32, tag="lnn")
                nc.scalar.activation(lnn, nsq, AF.Ln)
                s_t = stats.tile([128, NC], F32, tag="s")
                nc.scalar.activation(s_t, lnn, AF.Exp, scale=-0.25)
                rtn = stats.tile([128, NC], F32, tag="rtn")
                nc.scalar.activation(rtn, lnn, AF.Exp, scale=0.25)

                # ---- ut rows: [128, NC, 65]; col 64 = s ------------------------
                ut = prep.tile([128, NC, DA], F32, tag="ut")
                for c in range(NC):
                    nc.vector.tensor_scalar(
                        out=ut[:, c, 0:D], in0=x_sb[:, c, :],
                        scalar1=s_t[:, c:c + 1], scalar2=None, op0=OP.mult,
                    )
                nc.vector.tensor_copy(ut[:, :, D], s_t)

                # ---- transpose ut chunks -> UT [65, 1024] ----------------------
                utT = big.tile([DA, S], F32R, tag="utT")
                for c in range(NC):
                    pt = psm.tile([DA, 128], F32, tag="small")
                    nc.tensor.transpose(pt, ut[:, c, :], ident)
                    if c % 2 == 0:
                        nc.vector.tensor_copy(utT[:, c * 128:(c + 1) * 128], pt)
                    else:
                        nc.scalar.copy(utT[:, c * 128:(c + 1) * 128], pt)

                # ---- Z^T = W^T UT  [64, 1024]  (f32r) --------------------------
                zT = big.tile([D, S], F32R, tag="zT")
                for h in range(2):
                    zp = psm.tile([D, 512], F32, tag="small")
                    nc.tensor.matmul(
                        zp, w_r, utT[0:D, h * 512:(h + 1) * 512],
                        start=True, stop=True,
                    )
                    if h == 0:
                        nc.vector.tensor_copy(zT[:, h * 512:(h + 1) * 512], zp)
                    else:
                        nc.scalar.copy(zT[:, h * 512:(h + 1) * 512], zp)

                # ---- V' rows: V'[j, e] = sqrt(n_j) Vt + bv ; col64 = 1 ---------
                v_sb = prep.tile([128, NC, DA], BF16, tag="v")
                for c in range(NC):
                    vp = psm.tile([128, DA], F32, tag="small")
                    # K = 65: rows 0..63 = Wv^T (zero col 64), row 64 = [bv, 1]
                    # (fp32: the f32r ISA mode rejects odd K/N sizes)
                    nc.tensor.matmul(
                        vp, utT[0:DA, c * 128:(c + 1) * 128].bitcast(F32), wv_sb,
                        start=True, stop=True,
                    )
                    nc.vector.tensor_scalar(
                        out=v_sb[:, c, :], in0=vp,
                        scalar1=rtn[:, c:c + 1], scalar2=None, op0=OP.mult,
                    )

                # ---- scores + softmax numerators + PV, chunk by chunk ----------
                po0 = pop.tile([DA, 512], F32, tag="po0")
                po1 = pop.tile([DA, 512], F32, tag="po1")
                for jc in range(NC):
                    gp = pgp.tile([128, S], F32, tag="g")
                    nc.tensor.matmul(
                        gp[:, 0:512],
                        zT[:, jc * 128:(jc + 1) * 128],
                        utT[0:D, 0:512], start=True, stop=True,
                    )
                    nc.tensor.matmul(
                        gp[:, 512:1024],
                        zT[:, jc * 128:(jc + 1) * 128],
                        utT[0:D, 512:1024], start=True, stop=True,
                    )
                    sq = score.tile([128, S], BF16, tag="sq")
                    if jc in ACT_SQUARE_CHUNKS:
                        nc.scalar.activation(sq, gp, AF.Square)
                    else:
                        gc = score.tile([128, S], BF16, tag="gc")
                        nc.vector.tensor_copy(gc, gp)
                        nc.vector.tensor_mul(sq, gc, gc)
                    p_t = score.tile([128, S], BF16, tag="p")
                    nc.scalar.activation(p_t, sq, AF.Exp, scale=INV_SQRT_D)
                    nc.tensor.matmul(
                        po0, v_sb[:, jc, :], p_t[:, 0:512],
                        start=(jc == 0), stop=(jc == NC - 1),
                    )
                    nc.tensor.matmul(
                        po1, v_sb[:, jc, :], p_t[:, 512:1024],
                        start=(jc == 0), stop=(jc == NC - 1),
                    )

                # ---- O = [PV | denom]^T  [65, 1024] -> row chunks --------------
                o_sb = big.tile([DA, S], F32, tag="o")
                nc.vector.tensor_copy(o_sb[:, 0:512], po0)
                nc.scalar.copy(o_sb[:, 512:1024], po1)

                hp = lnp.tile([128, NC, D], F32, tag="hp")
                epsd = stats.tile([128, NC], F32, tag="epsd")
                for c in range(NC):
                    ht = psm.tile([128, DA], F32, tag="small")
                    nc.tensor.transpose(
                        ht, o_sb[:, c * 128:(c + 1) * 128], ident[0:DA, 0:DA]
                    )
                    # h' = attn_numer + denom * x   (LayerNorm is scale-invariant)
                    xd = lnp.tile([128, D], F32, tag="xd")
                    nc.vector.tensor_scalar(
                        out=xd, in0=x_sb[:, c, :],
                        scalar1=ht[:, D:DA], scalar2=None, op0=OP.mult,
                    )
                    nc.vector.tensor_add(hp[:, c, :], ht[:, 0:D], xd)
                    nc.vector.tensor_scalar(
                        out=epsd[:, c:c + 1], in0=ht[:, D:DA],
                        scalar1=ht[:, D:DA], scalar2=LN_EPS,
                        op0=OP.mult, op1=OP.mult,
                    )

                # ---- LayerNorm over D ------------------------------------------
                bst = lnp.tile([128, NC, 6], F32, tag="bst")
                mv = lnp.tile([128, NC, 2], F32, tag="mv")
                for c in range(NC):
                    nc.vector.bn_stats(bst[:, c, :], hp[:, c, :])
                    nc.vector.bn_aggr(mv[:, c, :], bst[:, c, :])
                vpe = stats.tile([128, NC], F32, tag="vpe")
                nc.vector.tensor_add(vpe, mv[:, :, 1], epsd)
                lnv = stats.tile([128, NC], F32, tag="lnv")
                nc.scalar.activation(lnv, vpe, AF.Ln)
                rstd = stats.tile([128, NC], F32, tag="rstd")
                nc.scalar.activation(rstd, lnv, AF.Exp, scale=-0.5)

                o_rows = lnp.tile([128, NC, D], F32, tag="orows")
                for c in range(NC):
                    nc.vector.tensor_scalar(
                        out=o_rows[:, c, :], in0=hp[:, c, :],
                        scalar1=mv[:, c, 0:1], scalar2=rstd[:, c:c + 1],
                        op0=OP.subtract, op1=OP.mult,
                    )
                nc.sync.dma_start(
                    out_d[b].rearrange("(c p) d -> p c d", p=128), o_rows
                )
    nc.compile()
    return nc


_CACHED = None


def _get_nc():
    global _CACHED
    if _CACHED is None:
        _CACHED = _build_nc()
    return _CACHED


def kernel(x: np.ndarray, Wv: np.ndarray, bv: np.ndarray, phi: np.ndarray) -> np.ndarray:
    x = np.ascontiguousarray(np.asarray(x, np.float32))
    Wv = np.asarray(Wv, np.float32)
    bv = np.asarray(bv, np.float32)
    phi = np.asarray(phi, np.float32)
    assert x.shape == (B, S, D), x.shape

    wg = _givens_orthogonal(phi, D)
    wv_aug = np.zeros((DA, DA), np.float32)
    wv_aug[0:D, 0:D] = Wv.T
    wv_aug[D, 0:D] = bv
    wv_aug[D, D] = 1.0
    ident = np.eye(128, dtype=np.float32)

    nc = _get_nc()
    in_maps = [
        {
            "x": np.ascontiguousarray(x[c * NB:(c + 1) * NB]),
            "wg": wg,
            "wv_aug": wv_aug,
            "ident": ident,
        }
        for c in range(NCORES)
    ]
    res = run_bass_kernel_spmd(nc, in_maps, list(range(NCORES)))
    out = np.concatenate([res.results[c]["out"] for c in range(NCORES)], axis=0)
    return out.astype(np.float32)


if __name__ == "__main__":
    rng = np.random.default_rng(0)
    x = rng.standard_normal((B, S, D)).astype(np.float32)
    Wv = (rng.standard_normal((D, D)) / math.sqrt(D)).astype(np.float32)
    bv = (rng.standard_normal(D) * 0.01).astype(np.float32)
    phi = rng.uniform(0, 2 * math.pi, 2 * D - 3).astype(np.float32)
    y = kernel(x=x, Wv=Wv, bv=bv, phi=phi)
    print("out", y.shape, y.dtype, np.abs(y).mean())


# revision 11
# speedup vs baseline: 1.1031x; 1.1031x over previous
"""Trainium2 Bass kernel for nn_AttentionHead_Hybrid1 (quantum-inspired attention head).

Computation (per batch b of a [B=64, S=1024, D=64] input):
    n_i   = ||x_i||;  u_i = x_i / n_i
    W     = givens_orthogonal(phi)                (tiny, sequential -> host)
    A     = (n_i n_j) (u_i^T W^T u_j)^2           (S x S scores)
    V     = x Wv^T + bv
    out   = LayerNorm(softmax(A/sqrt(D)) V + x)

Kernel strategy (data-parallel over batch, 8 batches per NeuronCore):
  * Fold norms into the score matmul:  ut_i = x_i / sqrt(n_i),
    G^T = (ut W) ut^T, so A^T = G^T * G^T elementwise.
  * Scores computed directly in transposed [j, i] layout so the PV matmul
    needs no transpose of the softmax matrix.
  * Softmax denominator comes out of the PV matmul via an appended
    all-ones column of V (the column materializes automatically from the
    bias-fold trick: the augmented K=65 matmul adds s_j*[bv,1] rows and the
    later *sqrt(n_j) rescale turns column 64 into exactly 1).
  * No max-subtraction needed: max exponent of A/sqrt(D) is ~4.
  * The softmax division is eliminated entirely: LayerNorm is invariant to
    per-row scaling, so we feed it h' = (P V) + denom * x and scale the
    variance epsilon by denom^2.
  * 1/sqrt and x^(+-1/4) are computed as exp(c*ln(x)) so the only ACT table
    set needed is natural_log_exp_and_others (exp/ln/square/copy); the
    activation-table map is restricted to that set so walrus never thrashes
    table loads.
  * The whole matmul path runs in BF16 (measured end-to-end error ~1e-4):
    fp32/f32r matmuls lower to multi-pass fp32 mode at 2-4x the cost.
"""

import math
import sys

import numpy as np

sys.path.insert(0, "/opt/trn_rl_repo")

import concourse.bass as bass
import concourse.bacc as bacc
import concourse.tile as tile
from concourse import mybir
from concourse.bass_utils import run_bass_kernel_spmd

try:
    import ml_dtypes
    BF16_NP = ml_dtypes.bfloat16
except ImportError:  # pragma: no cover
    BF16_NP = None

F32 = mybir.dt.float32
BF16 = mybir.dt.bfloat16
AX = mybir.AxisListType.X
OP = mybir.AluOpType
AF = mybir.ActivationFunctionType

B, S, D = 64, 1024, 64
NCORES = 8
NB = B // NCORES          # batches per core
NC = S // 128             # 128-row chunks per batch
DA = D + 1                # V augmented with ones column
INV_SQRT_D = 1.0 / math.sqrt(D)
LN_EPS = 1e-5

# score chunks whose squaring runs on ScalarE instead of VectorE (load balance)
ACT_SQUARE_CHUNKS = frozenset({5, 6, 7})

_ACT_SET = "natural_log_exp_and_others"


def _patch_act_tables():
    """Make every activation resolve to one table set (it contains every
    function this kernel uses), so the compiled stream has exactly one
    ACT_TABLE_LOAD instead of ping-ponging between per-anchor sets."""
    from concourse import hw_specs

    if getattr(bacc, "_act_tables_patched", False):
        return
    orig = hw_specs.get_activation_tables

    def patched(arch):
        tabs = orig(arch)
        return {
            name: (funcs if name == _ACT_SET else set())
            for name, funcs in tabs.items()
        }

    bacc.get_activation_tables = patched
    bacc._act_tables_patched = True


def _givens_orthogonal(phi: np.ndarray, d: int) -> np.ndarray:
    pairs = [(i, i + 1) for i in range(d - 1)] + [(i, i + 1) for i in range(d - 3, -1, -1)]
    W = np.eye(d, dtype=np.float64)
    p = phi.astype(np.float64)
    for k, (i, j) in enumerate(pairs):
        c, s = np.cos(p[k]), np.sin(p[k])
        ri, rj = W[i].copy(), W[j].copy()
        W[i] = c * ri + s * rj
        W[j] = -s * ri + c * rj
    return W.astype(np.float32)


def _build_nc() -> bass.Bass:
    _patch_act_tables()
    nc = bacc.Bacc("TRN2", target_bir_lowering=False, debug=False, num_devices=NCORES)

    x_d = nc.dram_tensor("x", [NB, S, D], F32, kind="ExternalInput").ap()
    w_d = nc.dram_tensor("wg", [D, D], BF16, kind="ExternalInput").ap()
    wv_d = nc.dram_tensor("wv_aug", [DA, DA], BF16, kind="ExternalInput").ap()
    idb_d = nc.dram_tensor("ident_b", [128, 128], BF16, kind="ExternalInput").ap()
    idf_d = nc.dram_tensor("ident_f", [128, 128], F32, kind="ExternalInput").ap()
    out_d = nc.dram_tensor("out", [NB, S, D], F32, kind="ExternalOutput").ap()

    with tile.TileContext(nc) as tc:
        with (
            tc.tile_pool(name="const", bufs=1) as constp,
            tc.tile_pool(name="xin", bufs=2) as xin,
            tc.tile_pool(name="prep", bufs=2) as prep,
            tc.tile_pool(name="stats", bufs=2) as stats,
            tc.tile_pool(name="big", bufs=2) as big,
            tc.tile_pool(name="score", bufs=3) as score,
            tc.tile_pool(name="lnp", bufs=2) as lnp,
            tc.tile_pool(name="pg", bufs=2, space="PSUM") as pgp,
            tc.tile_pool(name="po", bufs=1, space="PSUM") as pop,
            tc.tile_pool(name="psm", bufs=2, space="PSUM") as psm,
        ):
            w_sb = constp.tile([D, D], BF16)
            nc.sync.dma_start(w_sb, w_d)
            wv_sb = constp.tile([DA, DA], BF16)
            nc.sync.dma_start(wv_sb, wv_d)
            ident_b = constp.tile([128, 128], BF16)
            nc.sync.dma_start(ident_b, idb_d)
            ident_f = constp.tile([128, 128], F32)
            nc.sync.dma_start(ident_f, idf_d)

            for b in range(NB):
                # ---- load x[b] as [128, NC, 64] --------------------------------
                x_sb = xin.tile([128, NC, D], F32)
                nc.sync.dma_start(x_sb, x_d[b].rearrange("(c p) d -> p c d", p=128))

                # ---- norms: nsq, s = nsq^-1/4, rtn = nsq^+1/4 ------------------
                xsq = prep.tile([128, NC, D], F32, tag="xsq")
                nc.gpsimd.tensor_mul(xsq, x_sb, x_sb)
                nsq = stats.tile([128, NC], F32, tag="nsq")
                nc.vector.reduce_sum(nsq, xsq, axis=AX)
                lnn = stats.tile([128, NC], F32, tag="lnn")
                nc.scalar.activation(lnn, nsq, AF.Ln)
                s_t = stats.tile([128, NC], F32, tag="s")
                nc.scalar.activation(s_t, lnn, AF.Exp, scale=-0.25)
                rtn = stats.tile([128, NC], F32, tag="rtn")
                nc.scalar.activation(rtn, lnn, AF.Exp, scale=0.25)

                # ---- ut rows (bf16): [128, NC, 65]; col 64 = s -----------------
                ut = prep.tile([128, NC, DA], BF16, tag="ut")
                for c in range(NC):
                    nc.vector.tensor_scalar(
                        out=ut[:, c, 0:D], in0=x_sb[:, c, :],
                        scalar1=s_t[:, c:c + 1], scalar2=None, op0=OP.mult,
                    )
                nc.vector.tensor_copy(ut[:, :, D], s_t)

                # ---- transpose ut chunks -> UT [65, 1024] bf16 -----------------
                utT = big.tile([DA, S], BF16, tag="utT")
                for c in range(NC):
                    pt = psm.tile([DA, 128], BF16, tag="small")
                    nc.tensor.transpose(pt, ut[:, c, :], ident_b)
                    if c % 2 == 0:
                        nc.vector.tensor_copy(utT[:, c * 128:(c + 1) * 128], pt)
                    else:
                        nc.scalar.copy(utT[:, c * 128:(c + 1) * 128], pt)

                # ---- Z^T = W^T UT  [64, 1024] bf16 -----------------------------
                zT = big.tile([D, S], BF16, tag="zT")
                for h in range(2):
                    zp = psm.tile([D, 512], F32, tag="small")
                    nc.tensor.matmul(
                        zp, w_sb, utT[0:D, h * 512:(h + 1) * 512],
                        start=True, stop=True,
                    )
                    if h == 0:
                        nc.vector.tensor_copy(zT[:, h * 512:(h + 1) * 512], zp)
                    else:
                        nc.scalar.copy(zT[:, h * 512:(h + 1) * 512], zp)

                # ---- V' rows: V'[j, e] = sqrt(n_j) Vt + bv ; col64 = 1 ---------
                v_sb = prep.tile([128, NC, DA], BF16, tag="v")
                for c in range(NC):
                    vp = psm.tile([128, DA], F32, tag="small")
                    # K = 65: rows 0..63 = Wv^T (zero col 64), row 64 = [bv, 1]
                    nc.tensor.matmul(
                        vp, utT[0:DA, c * 128:(c + 1) * 128], wv_sb,
                        start=True, stop=True,
                    )
                    nc.vector.tensor_scalar(
                        out=v_sb[:, c, :], in0=vp,
                        scalar1=rtn[:, c:c + 1], scalar2=None, op0=OP.mult,
                    )

                # ---- scores + softmax numerators + PV, chunk by chunk ----------
                po0 = pop.tile([DA, 512], F32, tag="po0")
                po1 = pop.tile([DA, 512], F32, tag="po1")
                for jc in range(NC):
                    gp = pgp.tile([128, S], F32, tag="g")
                    nc.tensor.matmul(
                        gp[:, 0:512],
                        zT[:, jc * 128:(jc + 1) * 128],
                        utT[0:D, 0:512], start=True, stop=True,
                    )
                    nc.tensor.matmul(
                        gp[:, 512:1024],
                        zT[:, jc * 128:(jc + 1) * 128],
                        utT[0:D, 512:1024], start=True, stop=True,
                    )
                    sq = score.tile([128, S], BF16, tag="sq")
                    if jc in ACT_SQUARE_CHUNKS:
                        nc.scalar.activation(sq, gp, AF.Square)
                    else:
                        gc = score.tile([128, S], BF16, tag="gc")
                        nc.vector.tensor_copy(gc, gp)
                        nc.vector.tensor_mul(sq, gc, gc)
                    p_t = score.tile([128, S], BF16, tag="p")
                    nc.scalar.activation(p_t, sq, AF.Exp, scale=INV_SQRT_D)
                    nc.tensor.matmul(
                        po0, v_sb[:, jc, :], p_t[:, 0:512],
                        start=(jc == 0), stop=(jc == NC - 1),
                    )
                    nc.tensor.matmul(
                        po1, v_sb[:, jc, :], p_t[:, 512:1024],
                        start=(jc == 0), stop=(jc == NC - 1),
                    )

                # ---- O = [PV | denom]^T  [65, 1024] fp32 -----------------------
                o_sb = big.tile([DA, S], F32, tag="o")
                nc.vector.tensor_copy(o_sb[:, 0:512], po0)
                nc.scalar.copy(o_sb[:, 512:1024], po1)

                hp = lnp.tile([128, NC, D], F32, tag="hp")
                epsd = stats.tile([128, NC], F32, tag="epsd")
                for c in range(NC):
                    ht = psm.tile([128, DA], F32, tag="small")
                    nc.tensor.transpose(
                        ht, o_sb[:, c * 128:(c + 1) * 128], ident_f[0:DA, 0:DA]
                    )
                    # h' = denom * x + attn_numer   (LayerNorm is scale-invariant)
                    nc.vector.scalar_tensor_tensor(
                        out=hp[:, c, :], in0=x_sb[:, c, :],
                        scalar=ht[:, D:DA], in1=ht[:, 0:D],
                        op0=OP.mult, op1=OP.add,
                    )
                    nc.vector.tensor_scalar(
                        out=epsd[:, c:c + 1], in0=ht[:, D:DA],
                        scalar1=ht[:, D:DA], scalar2=LN_EPS,
                        op0=OP.mult, op1=OP.mult,
                    )

                # ---- LayerNorm over D ------------------------------------------
                bst = lnp.tile([128, NC, 6], F32, tag="bst")
                mv = lnp.tile([128, NC, 2], F32, tag="mv")
                for c in range(NC):
                    nc.vector.bn_stats(bst[:, c, :], hp[:, c, :])
                    nc.vector.bn_aggr(mv[:, c, :], bst[:, c, :])
                vpe = stats.tile([128, NC], F32, tag="vpe")
                nc.vector.tensor_add(vpe, mv[:, :, 1], epsd)
                lnv = stats.tile([128, NC], F32, tag="lnv")
                nc.scalar.activation(lnv, vpe, AF.Ln)
                rstd = stats.tile([128, NC], F32, tag="rstd")
                nc.scalar.activation(rstd, lnv, AF.Exp, scale=-0.5)

                o_rows = lnp.tile([128, NC, D], F32, tag="orows")
                for c in range(NC):
                    nc.gpsimd.tensor_scalar(
                        out=o_rows[:, c, :], in0=hp[:, c, :],
                        scalar1=mv[:, c, 0:1], scalar2=rstd[:, c:c + 1],
                        op0=OP.subtract, op1=OP.mult,
                    )
                nc.sync.dma_start(
                    out_d[b].rearrange("(c p) d -> p c d", p=128), o_rows
                )
    nc.compile()
    return nc


_CACHED = None


def _get_nc():
    global _CACHED
    if _CACHED is None:
        _CACHED = _build_nc()
    return _CACHED


def _to_bf16(a: np.ndarray) -> np.ndarray:
    if BF16_NP is not None:
        return a.astype(BF16_NP)
    u = np.ascontiguousarray(a.astype(np.float32)).view(np.uint32)
    r = ((u >> 16) & 1).astype(np.uint32)
    return (((u + 0x7FFF + r) >> 16).astype(np.uint16)).view(np.uint16)


def kernel(x: np.ndarray, Wv: np.ndarray, bv: np.ndarray, phi: np.ndarray) -> np.ndarray:
    x = np.ascontiguousarray(np.asarray(x, np.float32))
    Wv = np.asarray(Wv, np.float32)
    bv = np.asarray(bv, np.float32)
    phi = np.asarray(phi, np.float32)
    assert x.shape == (B, S, D), x.shape

    wg = _givens_orthogonal(phi, D)
    wv_aug = np.zeros((DA, DA), np.float32)
    wv_aug[0:D, 0:D] = Wv.T
    wv_aug[D, 0:D] = bv
    wv_aug[D, D] = 1.0

    nc = _get_nc()
    in_maps = [
        {
            "x": np.ascontiguousarray(x[c * NB:(c + 1) * NB]),
            "wg": _to_bf16(wg),
            "wv_aug": _to_bf16(wv_aug),
            "ident_b": _to_bf16(np.eye(128, dtype=np.float32)),
            "ident_f": np.eye(128, dtype=np.float32),
        }
        for c in range(NCORES)
    ]
    res = run_bass_kernel_spmd(nc, in_maps, list(range(NCORES)))
    out = np.concatenate([res.results[c]["out"] for c in range(NCORES)], axis=0)
    return out.astype(np.float32)


if __name__ == "__main__":
    rng = np.random.default_rng(0)
    x = rng.standard_normal((B, S, D)).astype(np.float32)
    Wv = (rng.standard_normal((D, D)) / math.sqrt(D)).astype(np.float32)
    bv = (rng.standard_normal(D) * 0.01).astype(np.float32)
    phi = rng.uniform(0, 2 * math.pi, 2 * D - 3).astype(np.float32)
    y = kernel(x=x, Wv=Wv, bv=bv, phi=phi)
    print("out", y.shape, y.dtype, np.abs(y).mean())


# revision 13
# speedup vs baseline: 1.4104x; 1.2786x over previous
"""Trainium2 Bass kernel for nn_AttentionHead_Hybrid1 (quantum-inspired attention head).

Computation (per batch b of a [B=64, S=1024, D=64] input):
    n_i   = ||x_i||;  u_i = x_i / n_i
    W     = givens_orthogonal(phi)                (tiny, sequential -> host)
    A     = (n_i n_j) (u_i^T W^T u_j)^2           (S x S scores)
    V     = x Wv^T + bv
    out   = LayerNorm(softmax(A/sqrt(D)) V + x)

Kernel strategy (data-parallel over batch, 8 batches per NeuronCore):
  * Fold norms into the score matmul:  ut_i = x_i / sqrt(n_i),
    G^T = (ut W) ut^T, so A^T = G^T * G^T elementwise.
  * Scores computed directly in transposed [j, i] layout so the PV matmul
    needs no transpose of the softmax matrix.
  * Softmax denominator comes out of the PV matmul via an appended
    all-ones column of V; the sqrt(n_j) row-rescale of V is folded into the
    per-partition BIAS of the exp activation (exp(G^2/8 + ln sqrt(n_j))),
    making V' = ut Wv_aug^T directly usable and column 64 exactly 1/sqrt(n)
    whose rescale lands on the softmax denominator consistently.
  * No max-subtraction needed: max exponent of A/sqrt(D) is ~4.
  * The softmax division is eliminated entirely: LayerNorm is invariant to
    per-row scaling, so we feed it h' = (P V) + denom * x and scale the
    variance epsilon by denom^2.
  * 1/sqrt and x^(+-1/4) are computed as exp(c*ln(x)); the activation-table
    map is restricted to natural_log_exp_and_others so there is exactly one
    ACT table load in the whole kernel.
  * The whole matmul path runs in BF16 (measured end-to-end error ~1e-4).
  * Elementwise work is fused into few wide ops (free-dim-broadcast APs for
    per-row scalars) because per-instruction overhead dominates DVE/ACT.
"""

import math
import sys

import numpy as np

sys.path.insert(0, "/opt/trn_rl_repo")

import concourse.bass as bass
import concourse.bacc as bacc
import concourse.tile as tile
from concourse import mybir
from concourse.bass_utils import run_bass_kernel_spmd

try:
    import ml_dtypes
    BF16_NP = ml_dtypes.bfloat16
except ImportError:  # pragma: no cover
    BF16_NP = None

F32 = mybir.dt.float32
BF16 = mybir.dt.bfloat16
AX = mybir.AxisListType.X
OP = mybir.AluOpType
AF = mybir.ActivationFunctionType

B, S, D = 64, 1024, 64
NCORES = 8
NB = B // NCORES          # batches per core
NC = S // 128             # 128-row chunks per batch
DA = D + 1                # V augmented with ones column
INV_SQRT_D = 1.0 / math.sqrt(D)
LN_EPS = 1e-5

# score chunk pairs whose squaring runs on VectorE instead of ScalarE
DVE_SQUARE_PAIRS = frozenset({0})  # pairs: 0->(0,1) 1->(2,3) 2->(4,5) 3->(6,7)

_ACT_SET = "natural_log_exp_and_others"


def _patch_act_tables():
    """Make every activation resolve to one table set (it contains every
    function this kernel uses), so the compiled stream has exactly one
    ACT_TABLE_LOAD instead of ping-ponging between per-anchor sets."""
    from concourse import hw_specs

    if getattr(bacc, "_act_tables_patched", False):
        return
    orig = hw_specs.get_activation_tables

    def patched(arch):
        tabs = orig(arch)
        return {
            name: (funcs if name == _ACT_SET else set())
            for name, funcs in tabs.items()
        }

    bacc.get_activation_tables = patched
    bacc._act_tables_patched = True


def _givens_orthogonal(phi: np.ndarray, d: int) -> np.ndarray:
    pairs = [(i, i + 1) for i in range(d - 1)] + [(i, i + 1) for i in range(d - 3, -1, -1)]
    W = np.eye(d, dtype=np.float64)
    p = phi.astype(np.float64)
    for k, (i, j) in enumerate(pairs):
        c, s = np.cos(p[k]), np.sin(p[k])
        ri, rj = W[i].copy(), W[j].copy()
        W[i] = c * ri + s * rj
        W[j] = -s * ri + c * rj
    return W.astype(np.float32)


def _bcast_inner(ap, n):
    """[P, NC] -> [P, NC, n] with stride-0 inner dim."""
    return ap.unsqueeze(2).broadcast_to((ap.shape[0], ap.shape[1], n))


def _build_nc() -> bass.Bass:
    _patch_act_tables()
    nc = bacc.Bacc("TRN2", target_bir_lowering=False, debug=False, num_devices=NCORES)

    x_d = nc.dram_tensor("x", [NB, S, D], F32, kind="ExternalInput").ap()
    w_d = nc.dram_tensor("wg", [D, D], BF16, kind="ExternalInput").ap()
    wv_d = nc.dram_tensor("wv_aug", [DA, DA], BF16, kind="ExternalInput").ap()
    idb_d = nc.dram_tensor("ident_b", [128, 128], BF16, kind="ExternalInput").ap()
    idf_d = nc.dram_tensor("ident_f", [128, 128], F32, kind="ExternalInput").ap()
    out_d = nc.dram_tensor("out", [NB, S, D], F32, kind="ExternalOutput").ap()

    with tile.TileContext(nc) as tc:
        with (
            tc.tile_pool(name="const", bufs=1) as constp,
            tc.tile_pool(name="xin", bufs=3) as xin,
            tc.tile_pool(name="prep", bufs=2) as prep,
            tc.tile_pool(name="stats", bufs=2) as stats,
            tc.tile_pool(name="big", bufs=2) as big,
            tc.tile_pool(name="score", bufs=3) as score,
            tc.tile_pool(name="lnp", bufs=2) as lnp,
            tc.tile_pool(name="pbig", bufs=2, space="PSUM") as pbig,
            tc.tile_pool(name="po", bufs=1, space="PSUM") as pop,
            tc.tile_pool(name="psm", bufs=2, space="PSUM") as psm,
        ):
            w_sb = constp.tile([D, D], BF16)
            nc.sync.dma_start(w_sb, w_d)
            wv_sb = constp.tile([DA, DA], BF16)
            nc.sync.dma_start(wv_sb, wv_d)
            ident_b = constp.tile([128, 128], BF16)
            nc.sync.dma_start(ident_b, idb_d)
            ident_f = constp.tile([128, 128], F32)
            nc.sync.dma_start(ident_f, idf_d)

            for b in range(NB):
                # ---- load x[b] as [128, NC, 64] --------------------------------
                x_sb = xin.tile([128, NC, D], F32)
                nc.sync.dma_start(x_sb, x_d[b].rearrange("(c p) d -> p c d", p=128))

                # ---- norms: nsq; s = nsq^-1/4; lnq = ln(sqrt n) ----------------
                xsq = prep.tile([128, NC, D], F32, tag="xsq")
                nc.scalar.activation(xsq, x_sb, AF.Square)
                nsq = stats.tile([128, NC], F32, tag="nsq")
                nc.vector.reduce_sum(nsq, xsq, axis=AX)
                lnn = stats.tile([128, NC], F32, tag="lnn")
                nc.scalar.activation(lnn, nsq, AF.Ln)
                s_t = stats.tile([128, NC], F32, tag="s")
                nc.scalar.activation(s_t, lnn, AF.Exp, scale=-0.25)
                # exp bias: ln(sqrt n_j) = 0.25 * ln(nsq)
                lnq = stats.tile([128, NC], F32, tag="lnq")
                nc.vector.tensor_scalar(
                    out=lnq, in0=lnn, scalar1=0.25, scalar2=None, op0=OP.mult,
                )

                # ---- ut rows (bf16): [128, NC, 65]; col 64 = s -----------------
                ut = prep.tile([128, NC, DA], BF16, tag="ut")
                nc.vector.tensor_mul(
                    ut[:, :, 0:D], x_sb, _bcast_inner(s_t, D)
                )
                nc.vector.tensor_copy(ut[:, :, D], s_t)

                # ---- transpose ut chunks into ONE psum bank, 1 copy out --------
                ptall = psm.tile([DA, S], BF16, tag="small")
                for c in range(NC):
                    nc.tensor.transpose(ptall[:, c * 128:(c + 1) * 128], ut[:, c, :], ident_b)
                utT = big.tile([DA, S], BF16, tag="utT")
                nc.vector.tensor_copy(utT, ptall)

                # ---- Z^T = W^T UT  [64, 1024] bf16 -----------------------------
                zT = big.tile([D, S], BF16, tag="zT")
                for h in range(2):
                    zp = pbig.tile([D, 512], F32, tag="g")
                    nc.tensor.matmul(
                        zp, w_sb, utT[0:D, h * 512:(h + 1) * 512],
                        start=True, stop=True,
                    )
                    if h == 0:
                        nc.vector.tensor_copy(zT[:, h * 512:(h + 1) * 512], zp)
                    else:
                        nc.scalar.copy(zT[:, h * 512:(h + 1) * 512], zp)

                # ---- V'' = ut Wv_aug^T rows (no sqrt(n) rescale needed) --------
                # K = 65: rows 0..63 = Wv^T (zero col 64), row 64 = [bv, 1]
                vpall = pbig.tile([128, NC, 128], F32, tag="g")
                for c in range(NC):
                    nc.tensor.matmul(
                        vpall[:, c, 0:DA], utT[0:DA, c * 128:(c + 1) * 128], wv_sb,
                        start=True, stop=True,
                    )
                v_sb = prep.tile([128, NC, DA], BF16, tag="v")
                nc.vector.tensor_copy(v_sb, vpall[:, :, 0:DA])

                # ---- scores + softmax numerators + PV, 2 chunks per wave ------
                po0 = pop.tile([DA, 512], F32, tag="po0")
                po1 = pop.tile([DA, 512], F32, tag="po1")
                for w in range(NC // 2):
                    gps = []
                    for jc in (2 * w, 2 * w + 1):
                        gp = pbig.tile([128, S], F32, tag="g")
                        nc.tensor.matmul(
                            gp[:, 0:512],
                            zT[:, jc * 128:(jc + 1) * 128],
                            utT[0:D, 0:512], start=True, stop=True,
                        )
                        nc.tensor.matmul(
                            gp[:, 512:1024],
                            zT[:, jc * 128:(jc + 1) * 128],
                            utT[0:D, 512:1024], start=True, stop=True,
                        )
                        gps.append(gp)
                    sq = score.tile([128, 2, S], BF16, tag="sq")
                    if w in DVE_SQUARE_PAIRS:
                        gc = score.tile([128, 2, S], BF16, tag="gc")
                        nc.vector.tensor_copy(gc[:, 0, :], gps[0])
                        nc.vector.tensor_copy(gc[:, 1, :], gps[1])
                        nc.vector.tensor_mul(sq, gc, gc)
                    else:
                        nc.scalar.activation(sq[:, 0, :], gps[0], AF.Square)
                        nc.scalar.activation(sq[:, 1, :], gps[1], AF.Square)
                    p_t = score.tile([128, 2, S], BF16, tag="p")
                    for k, jc in enumerate((2 * w, 2 * w + 1)):
                        nc.scalar.activation(
                            p_t[:, k, :], sq[:, k, :], AF.Exp,
                            scale=INV_SQRT_D, bias=lnq[:, jc:jc + 1],
                        )
                        nc.tensor.matmul(
                            po0, v_sb[:, jc, :], p_t[:, k, 0:512],
                            start=(jc == 0), stop=(jc == NC - 1),
                        )
                        nc.tensor.matmul(
                            po1, v_sb[:, jc, :], p_t[:, k, 512:1024],
                            start=(jc == 0), stop=(jc == NC - 1),
                        )

                # ---- O = [PV | denom]^T  [65, 1024] fp32 -----------------------
                o_sb = big.tile([DA, S], F32, tag="o")
                nc.vector.tensor_copy(o_sb[:, 0:512], po0)
                nc.scalar.copy(o_sb[:, 512:1024], po1)

                # ---- transpose O chunks into one psum region -------------------
                htall = pbig.tile([128, NC, 128], F32, tag="g")
                for c in range(NC):
                    nc.tensor.transpose(
                        htall[:, c, 0:DA], o_sb[:, c * 128:(c + 1) * 128],
                        ident_f[0:DA, 0:DA],
                    )
                # denom row per chunk -> SBUF
                dn = lnp.tile([128, NC], F32, tag="dn")
                nc.vector.tensor_copy(dn, htall[:, :, D])
                # h' = denom * x + attn_numer  (LayerNorm scale-invariance)
                xd = lnp.tile([128, NC, D], F32, tag="xd")
                nc.vector.tensor_mul(xd, x_sb, _bcast_inner(dn, D))
                hp = lnp.tile([128, NC, D], F32, tag="hp")
                nc.vector.tensor_add(hp, xd, htall[:, :, 0:D])

                # ---- LayerNorm over D ------------------------------------------
                bst = lnp.tile([128, NC, 6], F32, tag="bst")
                mv = lnp.tile([128, NC, 2], F32, tag="mv")
                for c in range(NC):
                    nc.vector.bn_stats(bst[:, c, :], hp[:, c, :])
                    nc.vector.bn_aggr(mv[:, c, :], bst[:, c, :])
                dn2 = lnp.tile([128, NC], F32, tag="dn2")
                nc.vector.tensor_mul(dn2, dn, dn)
                vpe = stats.tile([128, NC], F32, tag="vpe")
                nc.vector.scalar_tensor_tensor(
                    out=vpe, in0=dn2, scalar=LN_EPS, in1=mv[:, :, 1],
                    op0=OP.mult, op1=OP.add,
                )
                lnv = stats.tile([128, NC], F32, tag="lnv")
                nc.scalar.activation(lnv, vpe, AF.Ln)
                rstd = stats.tile([128, NC], F32, tag="rstd")
                nc.scalar.activation(rstd, lnv, AF.Exp, scale=-0.5)

                # ---- (h' - mean) * rstd, broadcast ops -------------------------
                hm = lnp.tile([128, NC, D], F32, tag="hm")
                nc.vector.tensor_tensor(
                    out=hm, in0=hp, in1=_bcast_inner(mv[:, :, 0], D),
                    op=OP.subtract,
                )
                o_rows = lnp.tile([128, NC, D], F32, tag="orows")
                nc.vector.tensor_mul(o_rows, hm, _bcast_inner(rstd, D))
                nc.sync.dma_start(
                    out_d[b].rearrange("(c p) d -> p c d", p=128), o_rows
                )
    nc.compile()
    return nc


_CACHED = None


def _get_nc():
    global _CACHED
    if _CACHED is None:
        _CACHED = _build_nc()
    return _CACHED


def _to_bf16(a: np.ndarray) -> np.ndarray:
    if BF16_NP is not None:
        return a.astype(BF16_NP)
    u = np.ascontiguousarray(a.astype(np.float32)).view(np.uint32)
    r = ((u >> 16) & 1).astype(np.uint32)
    return (((u + 0x7FFF + r) >> 16).astype(np.uint16)).view(np.uint16)


def kernel(x: np.ndarray, Wv: np.ndarray, bv: np.ndarray, phi: np.ndarray) -> np.ndarray:
    x = np.ascontiguousarray(np.asarray(x, np.float32))
    Wv = np.asarray(Wv, np.float32)
    bv = np.asarray(bv, np.float32)
    phi = np.asarray(phi, np.float32)
    assert x.shape == (B, S, D), x.shape

    wg = _givens_orthogonal(phi, D)
    wv_aug = np.zeros((DA, DA), np.float32)
    wv_aug[0:D, 0:D] = Wv.T
    wv_aug[D, 0:D] = bv
    wv_aug[D, D] = 1.0

    nc = _get_nc()
    in_maps = [
        {
            "x": np.ascontiguousarray(x[c * NB:(c + 1) * NB]),
            "wg": _to_bf16(wg),
            "wv_aug": _to_bf16(wv_aug),
            "ident_b": _to_bf16(np.eye(128, dtype=np.float32)),
            "ident_f": np.eye(128, dtype=np.float32),
        }
        for c in range(NCORES)
    ]
    res = run_bass_kernel_spmd(nc, in_maps, list(range(NCORES)))
    out = np.concatenate([res.results[c]["out"] for c in range(NCORES)], axis=0)
    return out.astype(np.float32)


if __name__ == "__main__":
    rng = np.random.default_rng(0)
    x = rng.standard_normal((B, S, D)).astype(np.float32)
    Wv = (rng.standard_normal((D, D)) / math.sqrt(D)).astype(np.float32)
    bv = (rng.standard_normal(D) * 0.01).astype(np.float32)
    phi = rng.uniform(0, 2 * math.pi, 2 * D - 3).astype(np.float32)
    y = kernel(x=x, Wv=Wv, bv=bv, phi=phi)
    print("out", y.shape, y.dtype, np.abs(y).mean())


# revision 14
# speedup vs baseline: 1.5725x; 1.1149x over previous
"""Trainium2 Bass kernel for nn_AttentionHead_Hybrid1 (quantum-inspired attention head).

Computation (per batch b of a [B=64, S=1024, D=64] input):
    n_i   = ||x_i||;  u_i = x_i / n_i
    W     = givens_orthogonal(phi)                (tiny, sequential -> host)
    A     = (n_i n_j) (u_i^T W^T u_j)^2           (S x S scores)
    V     = x Wv^T + bv
    out   = LayerNorm(softmax(A/sqrt(D)) V + x)

Kernel strategy (data-parallel over batch, 8 batches per NeuronCore):
  * Fold norms into the score matmul:  ut_i = x_i / sqrt(n_i),
    G^T = (ut W) ut^T, so A^T = G^T * G^T elementwise.
  * Scores computed directly in transposed [j, i] layout so the PV matmul
    needs no transpose of the softmax matrix.
  * Softmax denominator comes out of the PV matmul via an appended
    all-ones column of V; the sqrt(n_j) row-rescale of V is folded into the
    per-partition BIAS of the exp activation (exp(G^2/8 + ln sqrt(n_j))),
    making V' = ut Wv_aug^T directly usable and column 64 exactly 1/sqrt(n)
    whose rescale lands on the softmax denominator consistently.
  * No max-subtraction needed: max exponent of A/sqrt(D) is ~4.
  * The softmax division is eliminated entirely: LayerNorm is invariant to
    per-row scaling, so we feed it h' = (P V) + denom * x and scale the
    variance epsilon by denom^2.
  * 1/sqrt and x^(+-1/4) are computed as exp(c*ln(x)); the activation-table
    map is restricted to natural_log_exp_and_others so there is exactly one
    ACT table load in the whole kernel.
  * The whole matmul path runs in BF16 (measured end-to-end error ~1e-4).
  * Elementwise work is fused into few wide ops (free-dim-broadcast APs for
    per-row scalars) because per-instruction overhead dominates DVE/ACT.
"""

import math
import sys

import numpy as np

sys.path.insert(0, "/opt/trn_rl_repo")

import concourse.bass as bass
import concourse.bacc as bacc
import concourse.tile as tile
from concourse import mybir
from concourse.bass_utils import run_bass_kernel_spmd

try:
    import ml_dtypes
    BF16_NP = ml_dtypes.bfloat16
except ImportError:  # pragma: no cover
    BF16_NP = None

F32 = mybir.dt.float32
BF16 = mybir.dt.bfloat16
AX = mybir.AxisListType.X
OP = mybir.AluOpType
AF = mybir.ActivationFunctionType

B, S, D = 64, 1024, 64
NCORES = 8
NB = B // NCORES          # batches per core
NC = S // 128             # 128-row chunks per batch
DA = D + 1                # V augmented with ones column
INV_SQRT_D = 1.0 / math.sqrt(D)
LN_EPS = 1e-5

# score chunk pairs whose squaring runs on VectorE instead of ScalarE
DVE_SQUARE_PAIRS = frozenset({0, 2})  # pairs: 0->(0,1) 1->(2,3) 2->(4,5) 3->(6,7)

_ACT_SET = "natural_log_exp_and_others"


def _patch_act_tables():
    """Make every activation resolve to one table set (it contains every
    function this kernel uses), so the compiled stream has exactly one
    ACT_TABLE_LOAD instead of ping-ponging between per-anchor sets."""
    from concourse import hw_specs

    if getattr(bacc, "_act_tables_patched", False):
        return
    orig = hw_specs.get_activation_tables

    def patched(arch):
        tabs = orig(arch)
        return {
            name: (funcs if name == _ACT_SET else set())
            for name, funcs in tabs.items()
        }

    bacc.get_activation_tables = patched
    bacc._act_tables_patched = True


def _givens_orthogonal(phi: np.ndarray, d: int) -> np.ndarray:
    pairs = [(i, i + 1) for i in range(d - 1)] + [(i, i + 1) for i in range(d - 3, -1, -1)]
    W = np.eye(d, dtype=np.float64)
    p = phi.astype(np.float64)
    for k, (i, j) in enumerate(pairs):
        c, s = np.cos(p[k]), np.sin(p[k])
        ri, rj = W[i].copy(), W[j].copy()
        W[i] = c * ri + s * rj
        W[j] = -s * ri + c * rj
    return W.astype(np.float32)


def _bcast_inner(ap, n):
    """[P, NC] -> [P, NC, n] with stride-0 inner dim."""
    return ap.unsqueeze(2).broadcast_to((ap.shape[0], ap.shape[1], n))


def _build_nc() -> bass.Bass:
    _patch_act_tables()
    nc = bacc.Bacc("TRN2", target_bir_lowering=False, debug=False, num_devices=NCORES)

    x_d = nc.dram_tensor("x", [NB, S, D], F32, kind="ExternalInput").ap()
    w_d = nc.dram_tensor("wg", [D, D], BF16, kind="ExternalInput").ap()
    wv_d = nc.dram_tensor("wv_aug", [DA, DA], BF16, kind="ExternalInput").ap()
    idb_d = nc.dram_tensor("ident_b", [128, 128], BF16, kind="ExternalInput").ap()
    idf_d = nc.dram_tensor("ident_f", [128, 128], F32, kind="ExternalInput").ap()
    out_d = nc.dram_tensor("out", [NB, S, D], F32, kind="ExternalOutput").ap()

    with tile.TileContext(nc) as tc:
        with (
            tc.tile_pool(name="const", bufs=1) as constp,
            tc.tile_pool(name="xin", bufs=3) as xin,
            tc.tile_pool(name="prep", bufs=2) as prep,
            tc.tile_pool(name="stats", bufs=2) as stats,
            tc.tile_pool(name="big", bufs=2) as big,
            tc.tile_pool(name="score", bufs=3) as score,
            tc.tile_pool(name="lnp", bufs=2) as lnp,
            tc.tile_pool(name="pbig", bufs=3, space="PSUM") as pbig,
            tc.tile_pool(name="po", bufs=1, space="PSUM") as pop,
        ):
            w_sb = constp.tile([D, D], BF16)
            nc.sync.dma_start(w_sb, w_d)
            wv_sb = constp.tile([DA, DA], BF16)
            nc.sync.dma_start(wv_sb, wv_d)
            ident_b = constp.tile([128, 128], BF16)
            nc.sync.dma_start(ident_b, idb_d)
            ident_f = constp.tile([128, 128], F32)
            nc.sync.dma_start(ident_f, idf_d)

            for b in range(NB):
                # ---- load x[b] as [128, NC, 64] --------------------------------
                x_sb = xin.tile([128, NC, D], F32)
                nc.sync.dma_start(x_sb, x_d[b].rearrange("(c p) d -> p c d", p=128))

                # ---- norms: nsq; s = nsq^-1/4; lnq = ln(sqrt n) ----------------
                xsq = prep.tile([128, NC, D], F32, tag="xsq")
                nc.scalar.activation(xsq, x_sb, AF.Square)
                nsq = stats.tile([128, NC], F32, tag="nsq")
                nc.vector.reduce_sum(nsq, xsq, axis=AX)
                lnn = stats.tile([128, NC], F32, tag="lnn")
                nc.scalar.activation(lnn, nsq, AF.Ln)
                s_t = stats.tile([128, NC], F32, tag="s")
                nc.scalar.activation(s_t, lnn, AF.Exp, scale=-0.25)
                # exp bias: ln(sqrt n_j) = 0.25 * ln(nsq)
                lnq = stats.tile([128, NC], F32, tag="lnq")
                nc.vector.tensor_scalar(
                    out=lnq, in0=lnn, scalar1=0.25, scalar2=None, op0=OP.mult,
                )

                # ---- ut rows (bf16): [128, NC, 65]; col 64 = s -----------------
                ut = prep.tile([128, NC, DA], BF16, tag="ut")
                nc.vector.tensor_mul(
                    ut[:, :, 0:D], x_sb, _bcast_inner(s_t, D)
                )
                nc.vector.tensor_copy(ut[:, :, D], s_t)

                # ---- transpose ut chunks into ONE psum bank, 1 copy out --------
                ptall = pbig.tile([DA, S], BF16, tag="g")
                for c in range(NC):
                    nc.tensor.transpose(ptall[:, c * 128:(c + 1) * 128], ut[:, c, :], ident_b)
                utT = big.tile([DA, S], BF16, tag="utT")
                nc.vector.tensor_copy(utT, ptall)

                # ---- Z^T = W^T UT  [64, 1024] bf16 -----------------------------
                zT = big.tile([D, S], BF16, tag="zT")
                for h in range(2):
                    zp = pbig.tile([D, 512], F32, tag="g")
                    nc.tensor.matmul(
                        zp, w_sb, utT[0:D, h * 512:(h + 1) * 512],
                        start=True, stop=True,
                    )
                    if h == 0:
                        nc.vector.tensor_copy(zT[:, h * 512:(h + 1) * 512], zp)
                    else:
                        nc.scalar.copy(zT[:, h * 512:(h + 1) * 512], zp)

                # ---- V'' = ut Wv_aug^T rows (no sqrt(n) rescale needed) --------
                # K = 65: rows 0..63 = Wv^T (zero col 64), row 64 = [bv, 1]
                vpall = pbig.tile([128, NC, 128], F32, tag="g")
                for c in range(NC):
                    nc.tensor.matmul(
                        vpall[:, c, 0:DA], utT[0:DA, c * 128:(c + 1) * 128], wv_sb,
                        start=True, stop=True,
                    )
                v_sb = prep.tile([128, NC, DA], BF16, tag="v")
                nc.vector.tensor_copy(v_sb, vpall[:, :, 0:DA])

                # ---- scores + softmax numerators + PV, 2 chunks per wave ------
                po0 = pop.tile([DA, 512], F32, tag="po0")
                po1 = pop.tile([DA, 512], F32, tag="po1")
                for w in range(NC // 2):
                    gps = []
                    for jc in (2 * w, 2 * w + 1):
                        gp = pbig.tile([128, S], F32, tag="g")
                        nc.tensor.matmul(
                            gp[:, 0:512],
                            zT[:, jc * 128:(jc + 1) * 128],
                            utT[0:D, 0:512], start=True, stop=True,
                        )
                        nc.tensor.matmul(
                            gp[:, 512:1024],
                            zT[:, jc * 128:(jc + 1) * 128],
                            utT[0:D, 512:1024], start=True, stop=True,
                        )
                        gps.append(gp)
                    sq = score.tile([128, 2, S], BF16, tag="sq")
                    if w in DVE_SQUARE_PAIRS:
                        gc = score.tile([128, 2, S], BF16, tag="gc")
                        nc.vector.tensor_copy(gc[:, 0, :], gps[0])
                        nc.vector.tensor_copy(gc[:, 1, :], gps[1])
                        nc.vector.tensor_mul(sq, gc, gc)
                    else:
                        nc.scalar.activation(sq[:, 0, :], gps[0], AF.Square)
                        nc.scalar.activation(sq[:, 1, :], gps[1], AF.Square)
                    p_t = score.tile([128, 2, S], BF16, tag="p")
                    for k, jc in enumerate((2 * w, 2 * w + 1)):
                        nc.scalar.activation(
                            p_t[:, k, :], sq[:, k, :], AF.Exp,
                            scale=INV_SQRT_D, bias=lnq[:, jc:jc + 1],
                        )
                        nc.tensor.matmul(
                            po0, v_sb[:, jc, :], p_t[:, k, 0:512],
                            start=(jc == 0), stop=(jc == NC - 1),
                        )
                        nc.tensor.matmul(
                            po1, v_sb[:, jc, :], p_t[:, k, 512:1024],
                            start=(jc == 0), stop=(jc == NC - 1),
                        )

                # ---- O = [PV | denom]^T  [65, 1024] fp32 -----------------------
                o_sb = big.tile([DA, S], F32, tag="o")
                nc.vector.tensor_copy(o_sb[:, 0:512], po0)
                nc.scalar.copy(o_sb[:, 512:1024], po1)

                # ---- transpose O chunks into one psum region -------------------
                htall = pbig.tile([128, NC, 128], F32, tag="g")
                for c in range(NC):
                    nc.tensor.transpose(
                        htall[:, c, 0:DA], o_sb[:, c * 128:(c + 1) * 128],
                        ident_f[0:DA, 0:DA],
                    )
                # denom row per chunk -> SBUF
                dn = lnp.tile([128, NC], F32, tag="dn")
                nc.vector.tensor_copy(dn, htall[:, :, D])
                # h' = denom * x + attn_numer  (LayerNorm scale-invariance)
                xd = lnp.tile([128, NC, D], F32, tag="xd")
                nc.vector.tensor_mul(xd, x_sb, _bcast_inner(dn, D))
                hp = lnp.tile([128, NC, D], F32, tag="hp")
                nc.vector.tensor_add(hp, xd, htall[:, :, 0:D])

                # ---- LayerNorm over D ------------------------------------------
                bst = lnp.tile([128, NC, 6], F32, tag="bst")
                mv = lnp.tile([128, NC, 2], F32, tag="mv")
                for c in range(NC):
                    nc.vector.bn_stats(bst[:, c, :], hp[:, c, :])
                    nc.vector.bn_aggr(mv[:, c, :], bst[:, c, :])
                dn2 = lnp.tile([128, NC], F32, tag="dn2")
                nc.vector.tensor_mul(dn2, dn, dn)
                vpe = stats.tile([128, NC], F32, tag="vpe")
                nc.vector.scalar_tensor_tensor(
                    out=vpe, in0=dn2, scalar=LN_EPS, in1=mv[:, :, 1],
                    op0=OP.mult, op1=OP.add,
                )
                lnv = stats.tile([128, NC], F32, tag="lnv")
                nc.scalar.activation(lnv, vpe, AF.Ln)
                rstd = stats.tile([128, NC], F32, tag="rstd")
                nc.scalar.activation(rstd, lnv, AF.Exp, scale=-0.5)

                # ---- (h' - mean) * rstd, broadcast ops -------------------------
                hm = lnp.tile([128, NC, D], F32, tag="hm")
                nc.vector.tensor_tensor(
                    out=hm, in0=hp, in1=_bcast_inner(mv[:, :, 0], D),
                    op=OP.subtract,
                )
                o_rows = lnp.tile([128, NC, D], F32, tag="orows")
                nc.vector.tensor_mul(o_rows, hm, _bcast_inner(rstd, D))
                nc.sync.dma_start(
                    out_d[b].rearrange("(c p) d -> p c d", p=128), o_rows
                )
    nc.compile()
    return nc


_CACHED = None


def _get_nc():
    global _CACHED
    if _CACHED is None:
        _CACHED = _build_nc()
    return _CACHED


def _to_bf16(a: np.ndarray) -> np.ndarray:
    if BF16_NP is not None:
        return a.astype(BF16_NP)
    u = np.ascontiguousarray(a.astype(np.float32)).view(np.uint32)
    r = ((u >> 16) & 1).astype(np.uint32)
    return (((u + 0x7FFF + r) >> 16).astype(np.uint16)).view(np.uint16)


def kernel(x: np.ndarray, Wv: np.ndarray, bv: np.ndarray, phi: np.ndarray) -> np.ndarray:
    x = np.ascontiguousarray(np.asarray(x, np.float32))
    Wv = np.asarray(Wv, np.float32)
    bv = np.asarray(bv, np.float32)
    phi = np.asarray(phi, np.float32)
    assert x.shape == (B, S, D), x.shape

    wg = _givens_orthogonal(phi, D)
    wv_aug = np.zeros((DA, DA), np.float32)
    wv_aug[0:D, 0:D] = Wv.T
    wv_aug[D, 0:D] = bv
    wv_aug[D, D] = 1.0

    nc = _get_nc()
    in_maps = [
        {
            "x": np.ascontiguousarray(x[c * NB:(c + 1) * NB]),
            "wg": _to_bf16(wg),
            "wv_aug": _to_bf16(wv_aug),
            "ident_b": _to_bf16(np.eye(128, dtype=np.float32)),
            "ident_f": np.eye(128, dtype=np.float32),
        }
        for c in range(NCORES)
    ]
    res = run_bass_kernel_spmd(nc, in_maps, list(range(NCORES)))
    out = np.concatenate([res.results[c]["out"] for c in range(NCORES)], axis=0)
    return out.astype(np.float32)


if __name__ == "__main__":
    rng = np.random.default_rng(0)
    x = rng.standard_normal((B, S, D)).astype(np.float32)
    Wv = (rng.standard_normal((D, D)) / math.sqrt(D)).astype(np.float32)
    bv = (rng.standard_normal(D) * 0.01).astype(np.float32)
    phi = rng.uniform(0, 2 * math.pi, 2 * D - 3).astype(np.float32)
    y = kernel(x=x, Wv=Wv, bv=bv, phi=phi)
    print("out", y.shape, y.dtype, np.abs(y).mean())
